# revision 14
# baseline (speedup 1.0000x reference)
"""Cross-attention kernel for Trainium2 (8 NeuronCores, data-parallel over batch).

Problem (hardcoded): B=8, Sq=4096, Sk=77, E=1024, C=768, H=16 heads, D=64.

    q = x @ wq + bq; k = y @ wk + bk; v = y @ wv + bv
    out = softmax(q k^T / sqrt(D)) v @ wo + bo

Sharding: batch element b -> core b. No collectives.

All matmul operands are bf16 (1 cycle/row PE rate); PSUM accumulation fp32.

Key performance facts (measured on HW):
  - Any custom-ucode DVE op (e.g. reciprocal_approx_fast) in the NEFF drops
    the PE clock from 2.4 to 2.0 GHz for the WHOLE run (454 vs 379 ns dur on
    N=512 matmuls; 257 vs 216 ns issue spacing). The softmax reciprocal is
    therefore computed with STANDARD DVE ops: seed = ~bits(den) (bitwise-xor
    -1 of the fp32 pattern) then one Newton step, all fused into 3
    tensor_scalar/scalar_tensor_tensor ops + the eviction multiply
    (max rel err ~1.7e-3, measured).
  - K<128 matmul pairs on distinct PE row/col groups (tile_position) run
    concurrently; scores (K=64, rows 0/64) and pav/den (M=64, cols 0/64)
    pairs each retire in one N=512 slot.
  - Per-MM issue floor is N/2.4GHz + ~3ns; LDWEIGHTS is fully hidden.

Per-core pipeline (all matmuls contract over the SBUF partition dim):
  - Activations feature-major: xT[E, Sq] prepared host-side; 1/sqrt(D)
    folded into wq/bq host-side.
  - Startup: xT0/wq arrive as fine-grained DMA slices so the first QT
    matmul issues at ~2us (was ~17us): QT(0) groups 0-1 first, then
    phase 0 (kT via wk tiles, V via yT tiles), then the main loop; chunk-0
    fillers = QT(0) groups 2-7 + QT(1) groups 0-7 keep the PE busy during
    the scores->exp->attnV ScalarE round-trips.
  - scores^T[Sk, q] per head pair: two K=64 matmuls on PE row groups.
  - exp on ScalarE (no max-subtraction: scores are O(6), fp32 PSUM in,
    bf16 out).
  - attn@V + denominators col-paired into PSUM banks as in the baseline.
  - Normalization: notx = den ^ -1 (bitwise), m = den*S0*notx,
    r = (m-S1)*notx, then oT = pav * (-S0) * r fused into the PSUM
    eviction (DVE scalar_tensor_tensor).
  - out[q, E] = matmul(lhsT=oT tiles, rhs=wo tiles), bo added during
    eviction (DVE), fp32 out.
"""

import os
from contextlib import ExitStack

import numpy as np
import ml_dtypes

import concourse.bass as bass
import concourse.tile as tile
from concourse import bacc, mybir
from concourse.bass_utils import run_bass_kernel_spmd

N_CORES = 8
SQ = 4096
SK = 77
SKP = 80  # SK padded for kT psum tiles
E = 1024
C = 768
H = 16
D = 64
CHUNK = 512
NCHUNK = SQ // CHUNK  # 8
ET = E // 128  # 8 e-tiles
CT = C // 128  # 6 c-tiles
F32 = mybir.dt.float32
I32 = mybir.dt.int32
BF16 = mybir.dt.bfloat16
BF = ml_dtypes.bfloat16

# Newton-reciprocal constants (same as concourse reciprocal_approx_fast seed)
RS0 = -0.23549792
RS1 = 2.0017324

_PROGRAM = None


def _build_program():
    nc = bacc.Bacc(
        "TRN2", target_bir_lowering=False, debug=False, num_devices=N_CORES
    )
    # xT pre-tiled host-side: [chunk, partition, e-tile, col].
    xT_d = nc.dram_tensor(
        "xT", [NCHUNK, 128, ET, CHUNK], BF16, kind="ExternalInput"
    ).ap()
    yT_d = nc.dram_tensor("yT", [C, SKP], BF16, kind="ExternalInput").ap()
    wq_d = nc.dram_tensor("wq", [E, E], BF16, kind="ExternalInput").ap()
    bq_d = nc.dram_tensor("bq", [E], F32, kind="ExternalInput").ap()
    wk_d = nc.dram_tensor("wk", [C, E], BF16, kind="ExternalInput").ap()
    bk_d = nc.dram_tensor("bk", [E], F32, kind="ExternalInput").ap()
    wv_d = nc.dram_tensor("wv", [C, E], BF16, kind="ExternalInput").ap()
    bv_d = nc.dram_tensor("bv", [E], F32, kind="ExternalInput").ap()
    wo_d = nc.dram_tensor("wo", [E, E], BF16, kind="ExternalInput").ap()
    bo_d = nc.dram_tensor("bo", [E], F32, kind="ExternalInput").ap()
    out_d = nc.dram_tensor("out", [SQ, E], F32, kind="ExternalOutput").ap()

    wq_r = wq_d.rearrange("(t p) n -> p t n", p=128)
    wo_r = wo_d.rearrange("(t p) n -> p t n", p=128)
    yT_r = yT_d.rearrange("(t p) n -> p t n", p=128)
    wk_r = wk_d.rearrange("(t p) n -> p t n", p=128)
    wv_r = wv_d.rearrange("(t p) n -> p t n", p=128)

    with tile.TileContext(nc) as tc, ExitStack() as ctx, nc.allow_low_precision(
        reason="bf16 pipeline; fp32 PSUM accumulation throughout"
    ):
        consts = ctx.enter_context(tc.tile_pool(name="consts", bufs=1))
        wq_sb = consts.tile([128, ET, E], BF16)
        wo_sb = consts.tile([128, ET, E], BF16)
        kT_sb = consts.tile([128, ET, SKP], BF16)
        v_sb = consts.tile([SK, H * 64], BF16)
        ones_sb = consts.tile([SK, 64], BF16)
        bq_sb = consts.tile([128, ET], F32)
        bk_sb = consts.tile([128, ET], F32)
        bv_sb = consts.tile([SK, H * 64], F32)
        bo_sb = consts.tile([128, E], F32)

        nc.any.memset(ones_sb[:], 1.0)

        xT_pool = ctx.enter_context(tc.tile_pool(name="xT", bufs=3))
        qT_pool = ctx.enter_context(tc.tile_pool(name="qT", bufs=3))

        # --- Startup DMA stream -------------------------------------------
        # PE prefix order: kT (t-major over 8 PSUM banks) -> QT(0) (t-major
        # over 8 banks) -> V. Each t-step consumes exactly one wk/wq/xT0
        # t-slice, so the PE starts as soon as the first ~300KB lands and
        # stays paced by the DMA stream instead of waiting for whole
        # tensors. sync queue: yT, wk slices, xT0 slices, xT1, xT2, bo.
        # scalar queue: biases, wq slices, wv, wo.
        ph0 = ctx.enter_context(tc.tile_pool(name="ph0", bufs=1))
        yT_sb = ph0.tile([128, CT, SKP], BF16)
        wk_sb = ph0.tile([128, CT, E], BF16)
        wv_sb = ph0.tile([128, CT, E], BF16)
        xT0_sb = xT_pool.tile([128, ET, CHUNK], BF16, tag="xT")

        nc.sync.dma_start(yT_sb[:], yT_r)
        for lo, hi in ((0, 2), (2, 4), (4, 6)):
            nc.sync.dma_start(wk_sb[:, lo:hi, :], wk_r[:, lo:hi, :])
        for t in range(0, ET, 4):
            nc.sync.dma_start(
                xT0_sb[:, t : t + 4, :], xT_d[0][:, t : t + 4, :]
            )
        nc.scalar.dma_start(bq_sb[:], bq_d.rearrange("(t p) -> p t", p=128))
        nc.scalar.dma_start(bk_sb[:], bk_d.rearrange("(t p) -> p t", p=128))
        for lo, hi in ((0, 3), (3, 6), (6, 8)):
            nc.scalar.dma_start(wq_sb[:, lo:hi, :], wq_r[:, lo:hi, :])
        nc.scalar.dma_start(bv_sb[:], bv_d.partition_broadcast(SK))
        nc.scalar.dma_start(wv_sb[:], wv_r)

        # kT: t-major accumulation into 8 banks; one wk t-slice per step.
        with tc.tile_pool(name="ph_kt", bufs=8, space="PSUM") as ph_kt:
            psk = [ph_kt.tile([128, SKP], F32, tag="psk", name=f"psk{e}")
                   for e in range(ET)]
            for t in range(CT):
                for et in range(ET):
                    nc.tensor.matmul(
                        psk[et][:],
                        wk_sb[:, t, et * 128 : (et + 1) * 128],
                        yT_sb[:, t, :],
                        start=(t == 0),
                        stop=(t == CT - 1),
                    )
            for et in range(ET):
                nc.scalar.activation(
                    kT_sb[:, et, :],
                    psk[et][:],
                    mybir.ActivationFunctionType.Identity,
                    bias=bk_sb[:, et : et + 1],
                )

        # QT(0): t-major accumulation into 8 banks; one wq/xT0 t-slice per
        # step. Evictions (one per et) feed scores(0, et) in order.
        qT0_sb = qT_pool.tile([128, ET, CHUNK], BF16, tag="qT")
        with tc.tile_pool(name="ph_qt0", bufs=8, space="PSUM") as ph_qt0:
            psq0 = [ph_qt0.tile([128, CHUNK], F32, tag="psq0", name=f"psq0_{e}")
                    for e in range(ET)]
            for t in range(ET):
                for et in range(ET):
                    nc.tensor.matmul(
                        psq0[et][:],
                        wq_sb[:, t, et * 128 : (et + 1) * 128],
                        xT0_sb[:, t, :],
                        start=(t == 0),
                        stop=(t == ET - 1),
                    )
            for et in range(ET):
                nc.scalar.activation(
                    qT0_sb[:, et, :],
                    psq0[et][:],
                    mybir.ActivationFunctionType.Identity,
                    bias=bq_sb[:, et : et + 1],
                )

        # V (small PE cost, needed by the first attention tail).
        with tc.tile_pool(name="ph0v", bufs=2, space="PSUM") as ph0v:
            for g in range(2):
                psv = ph0v.tile([SK, CHUNK], F32, tag="psv")
                for t in range(CT):
                    nc.tensor.matmul(
                        psv[:],
                        yT_sb[:, t, 0:SK],
                        wv_sb[:, t, g * CHUNK : (g + 1) * CHUNK],
                        start=(t == 0),
                        stop=(t == CT - 1),
                    )
                nc.vector.tensor_tensor(
                    v_sb[:, g * CHUNK : (g + 1) * CHUNK],
                    psv[:],
                    bv_sb[:, g * CHUNK : (g + 1) * CHUNK],
                    mybir.AluOpType.add,
                )

        # --- Main-loop pools ---
        ps_q = ctx.enter_context(tc.tile_pool(name="ps_q", bufs=2, space="PSUM"))

        def emit_qt_group(xT_sb, qT_sb, et):
            ps = ps_q.tile([128, CHUNK], F32, tag="psq")
            for t in range(ET):
                nc.tensor.matmul(
                    ps[:],
                    wq_sb[:, t, et * 128 : (et + 1) * 128],
                    xT_sb[:, t, :],
                    start=(t == 0),
                    stop=(t == ET - 1),
                )
            # Alternate evictions between ScalarE and DVE so neither engine
            # jams the scores(c, 0) chunk-boundary dependency.
            if et % 2 == 0:
                nc.scalar.activation(
                    qT_sb[:, et, :],
                    ps[:],
                    mybir.ActivationFunctionType.Identity,
                    bias=bq_sb[:, et : et + 1],
                )
            else:
                nc.vector.tensor_scalar(
                    qT_sb[:, et, :], ps[:], bq_sb[:, et : et + 1], None,
                    mybir.AluOpType.add,
                )
        oT_pool = ctx.enter_context(tc.tile_pool(name="oT", bufs=2))
        exps_pool = ctx.enter_context(tc.tile_pool(name="exps", bufs=4))
        nr_pool = ctx.enter_context(tc.tile_pool(name="nr", bufs=6))
        outs_pool = ctx.enter_context(tc.tile_pool(name="outs", bufs=3))
        ps_s = ctx.enter_context(tc.tile_pool(name="ps_s", bufs=2, space="PSUM"))
        ps_pav = ctx.enter_context(tc.tile_pool(name="ps_pav", bufs=2, space="PSUM"))
        ps_den = ctx.enter_context(tc.tile_pool(name="ps_den", bufs=2, space="PSUM"))

        # wo needed first by final(0) during chunk 1; xT(1)/xT(2) by the
        # QT(1)/QT(2) fillers inside chunk 0.
        xT_tiles = {0: xT0_sb}
        qT_tiles = {0: qT0_sb}

        def load_xT(c):
            t_ = xT_pool.tile([128, ET, CHUNK], BF16, tag="xT", name="xTn")
            nc.sync.dma_start(t_[:], xT_d[c])
            xT_tiles[c] = t_

        load_xT(1)
        load_xT(2)
        nc.sync.dma_start(bo_sb[:], bo_d.partition_broadcast(128))
        for lo, hi in ((0, 4), (4, 8)):
            nc.scalar.dma_start(wo_sb[:, lo:hi, :], wo_r[:, lo:hi, :])

        def emit_scores(qT_sb, et):
            psa = ps_s.tile([SK, CHUNK], F32, tag="pss")
            psb = ps_s.tile([SK, CHUNK], F32, tag="pss")
            nc.tensor.matmul(
                psa[:], kT_sb[0:64, et, 0:SK], qT_sb[0:64, et, :],
                start=True, stop=True,
            )
            nc.tensor.matmul(
                psb[:], kT_sb[64:128, et, 0:SK], qT_sb[64:128, et, :],
                start=True, stop=True,
            )
            exa = exps_pool.tile([SK, CHUNK], BF16, tag="exps")
            exb = exps_pool.tile([SK, CHUNK], BF16, tag="exps")
            nc.scalar.activation(exa[:], psa[:], mybir.ActivationFunctionType.Exp)
            nc.scalar.activation(exb[:], psb[:], mybir.ActivationFunctionType.Exp)
            return exa, exb

        def emit_tail_att(oT_sb, exa, exb, et):
            hA, hB = 2 * et, 2 * et + 1
            pav = ps_pav.tile([128, CHUNK], F32, tag="pspav")
            nc.tensor.matmul(
                pav[0:64, :], v_sb[:, hA * 64 : (hA + 1) * 64], exa[:],
                start=True, stop=True,
            )
            nc.tensor.matmul(
                pav[64:128, :], v_sb[:, hB * 64 : (hB + 1) * 64], exb[:],
                start=True, stop=True,
            )
            den = ps_den.tile([128, CHUNK], F32, tag="psden")
            nc.tensor.matmul(
                den[0:64, :], ones_sb[:], exa[:], start=True, stop=True
            )
            nc.tensor.matmul(
                den[64:128, :], ones_sb[:], exb[:], start=True, stop=True
            )
            # Newton reciprocal with standard DVE ops (no custom ucode):
            #   notx = ~bits(den); m = den*RS0*notx; r = (m-RS1)*notx
            #   1/den ~= -RS0 * r   (folded into the eviction multiply)
            notx = nr_pool.tile([128, CHUNK], F32, tag="notx", name="notx")
            nc.vector.tensor_scalar(
                notx[:].bitcast(I32), den[:].bitcast(I32), -1, None,
                mybir.AluOpType.bitwise_xor,
            )
            m = nr_pool.tile([128, CHUNK], F32, tag="m", name="m")
            nc.vector.scalar_tensor_tensor(
                m[:], den[:], RS0, notx[:],
                mybir.AluOpType.mult, mybir.AluOpType.mult,
            )
            r = nr_pool.tile([128, CHUNK], F32, tag="r", name="r")
            nc.vector.scalar_tensor_tensor(
                r[:], m[:], RS1, notx[:],
                mybir.AluOpType.subtract, mybir.AluOpType.mult,
            )
            # Normalization fused into the PSUM eviction: oT = pav*(-RS0)*r.
            nc.vector.scalar_tensor_tensor(
                oT_sb[:, et, :], pav[:], -RS0, r[:],
                mybir.AluOpType.mult, mybir.AluOpType.mult,
            )

        # Final groups: two n-halves per qt-row accumulate into one
        # [128, 1024] tile; a single 512KB DMA per qt (alternating HWDGE
        # queues) replaces two 256KB ones — fewer queue slots, shorter
        # end-of-program semaphore drain.
        out_tiles = {}

        def emit_final_group(c, oT_sb, i):
            qt, half = i // 2, i % 2
            n0 = half * CHUNK
            ps = ps_q.tile([128, CHUNK], F32, tag="psq", name="psf")
            for t in range(ET):
                nc.tensor.matmul(
                    ps[:],
                    oT_sb[:, t, qt * 128 : (qt + 1) * 128],
                    wo_sb[:, t, n0 : n0 + CHUNK],
                    start=(t == 0),
                    stop=(t == ET - 1),
                )
            if half == 0:
                out_tiles[qt] = outs_pool.tile(
                    [128, E], F32, tag="osb", name="osb"
                )
            o_sb = out_tiles[qt]
            nc.vector.tensor_tensor(
                o_sb[:, n0 : n0 + CHUNK], ps[:], bo_sb[:, n0 : n0 + CHUNK],
                mybir.AluOpType.add,
            )
            if half == 1:
                r0 = c * CHUNK + qt * 128
                eng = nc.sync if qt % 2 == 0 else nc.scalar
                eng.dma_start(out_d[r0 : r0 + 128, :], o_sb[:])

        # Chunk-0 fillers: QT(1) then QT(2) groups, two per attention
        # iteration (QT(0) ran t-major in the prefix).
        qT_tiles[1] = qT_pool.tile([128, ET, CHUNK], BF16, tag="qT", name="qTn")
        qT_tiles[2] = qT_pool.tile([128, ET, CHUNK], BF16, tag="qT", name="qTn")
        fillers = [
            (lambda et=et: emit_qt_group(xT_tiles[1], qT_tiles[1], et))
            for et in range(ET)
        ] + [
            (lambda et=et: emit_qt_group(xT_tiles[2], qT_tiles[2], et))
            for et in range(ET)
        ]
        for c in range(NCHUNK):
            if 1 <= c <= NCHUNK - 3:
                load_xT(c + 2)
            qT_sb = qT_tiles[c]
            oT_sb = oT_pool.tile([128, ET, CHUNK], BF16, tag="oT")
            exs = [None] * ET
            for et in range(ET):
                # Final/filler groups first: at chunk boundaries scores(c, 0)
                # waits on the last qT(c) evictions (ScalarE); the final
                # group keeps the PE busy through that window.
                if c == 0:
                    for _ in range(2):
                        if fillers:
                            fillers.pop(0)()
                else:
                    emit_final_group(c - 1, prev_oT, et)
                exs[et] = emit_scores(qT_sb, et)
                if et >= 1:
                    emit_tail_att(oT_sb, *exs[et - 1], et - 1)
            emit_tail_att(oT_sb, *exs[ET - 1], ET - 1)
            prev_oT = oT_sb
            # QT for chunk c+2 (chunks 1-2 were filled during chunk 0).
            if 1 <= c <= NCHUNK - 3:
                qT_tiles[c + 2] = qT_pool.tile([128, ET, CHUNK], BF16, tag="qT", name="qTn")
                for et in range(ET):
                    emit_qt_group(xT_tiles[c + 2], qT_tiles[c + 2], et)
        # Tail: last chunk's output projection.
        for i in range(ET):
            emit_final_group(NCHUNK - 1, prev_oT, i)

    nc.compile()
    return nc


def _get_program():
    global _PROGRAM
    if _PROGRAM is None:
        _PROGRAM = _build_program()
    return _PROGRAM


def kernel(x, y, wq, bq, wk, bk, wv, bv, wo, bo):
    x = np.asarray(x, dtype=np.float32)
    y = np.asarray(y, dtype=np.float32)
    wq = np.asarray(wq, dtype=np.float32)
    bq = np.asarray(bq, dtype=np.float32)
    wk = np.asarray(wk, dtype=np.float32)
    bk = np.asarray(bk, dtype=np.float32)
    wv = np.asarray(wv, dtype=np.float32)
    bv = np.asarray(bv, dtype=np.float32)
    wo = np.asarray(wo, dtype=np.float32)
    bo = np.asarray(bo, dtype=np.float32)

    scale = np.float32(1.0 / np.sqrt(D))
    wq_s = (wq * scale).astype(BF)
    bq_s = (bq * scale).astype(np.float32)
    wk_b = wk.astype(BF)
    wv_b = wv.astype(BF)
    wo_b = wo.astype(BF)

    nc = _get_program()
    in_maps = []
    for b in range(N_CORES):
        # [E, Sq] -> [chunk, partition, e-tile, col], contiguous per chunk.
        xT = np.ascontiguousarray(
            x[b].T.reshape(ET, 128, NCHUNK, CHUNK).transpose(2, 1, 0, 3)
        ).astype(BF)
        yT = np.zeros((C, SKP), dtype=np.float32)
        yT[:, :SK] = y[b].T
        yT = yT.astype(BF)
        in_maps.append(
            {
                "xT": xT,
                "yT": yT,
                "wq": wq_s,
                "bq": bq_s,
                "wk": wk_b,
                "bk": bk.astype(np.float32),
                "wv": wv_b,
                "bv": bv.astype(np.float32),
                "wo": wo_b,
                "bo": bo,
            }
        )

    trace = bool(int(os.environ.get("KERNEL_TRACE", "0")))
    kwargs = {}
    if trace:
        kwargs = {"trace": True, "tmpdir": os.environ.get("KERNEL_TRACE_DIR")}
    try:
        res = run_bass_kernel_spmd(nc, in_maps, list(range(N_CORES)), **kwargs)
    except Exception:
        # The axon-tunneled devices occasionally report a transient
        # NRT_EXEC_UNIT_UNRECOVERABLE; a retry on the same executable has
        # been observed to succeed.
        res = run_bass_kernel_spmd(nc, in_maps, list(range(N_CORES)), **kwargs)
    if trace:
        kernel.last_exec_time_ns = res.exec_time_ns
        kernel.last_results = res
    out = np.stack([res.results[b]["out"] for b in range(N_CORES)])
    return np.ascontiguousarray(out)
